# revision 14
# baseline (speedup 1.0000x reference)
"""Trainium2 Bass kernel for multi-head attention (b=2, n=2048, dim=1024,
heads=16, dim_head=64) sharded over 8 NeuronCores.

Sharding: core c handles batch c//4 and head group c%4 (4 heads).  Each core
computes its heads' full attention plus their slice of the output projection
(rows of w_out), producing a partial [n, dim] output; the host sums the four
partials per batch.  No collectives.

On-device layout (per core, everything "transposed", tokens in the free dim):
  xT [1024, 2048]                    x^T for this batch
  QT/KT pair tiles [128, 2048]       two heads stacked on the partition dim
  S^T [128j, 512i] = K@Q^T           row-tiled concurrent matmuls (K=64 each)
  P^T = exp(S^T)                     mask pre-folded into K, scale into w_q
  out^T [128, 512] = [V|1]^T @ P^T   ones columns make rows 64:128 the softmax
                                     denominator, replicated 64x
  normT = out^T[0:64]/out^T[64:128]  per-head normalize (DVE divide)
  y[i, 1024] += normT_pair.T @ w_out_pair   K=128 full-array accumulation

All matmuls run as float32r (FP22 truncation, full PE rate at N>=512).
"""

import os
import sys

import numpy as np

for _p in ("/opt/trn_rl_repo",):
    if _p not in sys.path and os.path.isdir(_p):
        sys.path.insert(0, _p)

import concourse.bass as bass  # noqa: E402
import concourse.mybir as mybir  # noqa: E402
import concourse.tile as tile  # noqa: E402
from concourse import bacc  # noqa: E402
from concourse import bass_utils  # noqa: E402

F32 = mybir.dt.float32
F32R = mybir.dt.float32r
EXP = mybir.ActivationFunctionType.Exp
DIV = mybir.AluOpType.divide

B, N, DIM = 2, 2048, 1024
HEADS, DH = 16, 64
SCALE = DH ** -0.5
NCORES = 8
HPC = HEADS // (NCORES // B)  # heads per core = 4
NPAIRS = HPC // 2             # head pairs per core = 2

KC = DIM // 128               # contraction chunks for projections = 8
JC = N // 128                 # key chunks = 16
IQ = 4                        # query stripes
IQW = N // IQ                 # stripe width = 512
NB = N // 128                 # query blocks for the output projection = 16


def r(ap):
    return ap.bitcast(F32R)


def build_kernel(nc, tc, reps=1):
    xT = nc.dram_tensor("xT", [DIM, N], F32R, kind="ExternalInput").ap()
    maskrep = nc.dram_tensor("maskrep", [128, N], F32, kind="ExternalInput").ap()
    wq = nc.dram_tensor("wq", [DIM, HPC * DH], F32R, kind="ExternalInput").ap()
    wk = nc.dram_tensor("wk", [DIM, HPC * DH], F32R, kind="ExternalInput").ap()
    wv = nc.dram_tensor("wv", [DIM, HPC * DH], F32R, kind="ExternalInput").ap()
    wo = nc.dram_tensor("wo", [HPC * DH, DIM], F32R, kind="ExternalInput").ap()
    identd = nc.dram_tensor("identd", [128, 128], F32R, kind="ExternalInput").ap()
    onesd = nc.dram_tensor("onesd", [128, JC * 64], F32R, kind="ExternalInput").ap()
    y = nc.dram_tensor("y", [N, DIM], F32, kind="ExternalOutput").ap()

    with (
        tc.tile_pool(name="persist", bufs=1) as pers,
        tc.tile_pool(name="ptile", bufs=3) as ppool,
        tc.tile_pool(name="ysb", bufs=2) as ypool,
    ):
        # ---- persistent SBUF tensors ----
        qt = [pers.tile([128, N], F32R, tag=f"qt{p}", name=f"qt{p}") for p in range(NPAIRS)]
        kt = [pers.tile([128, N], F32R, tag=f"kt{p}", name=f"kt{p}") for p in range(NPAIRS)]
        v1 = [pers.tile([128, JC * 96], F32R, tag=f"v1{h}", name=f"v1{h}") for h in range(HPC)]
        nt = qt  # normalized out^T reuses the q tiles (stripe is dead post-attention)
        wo_sb = pers.tile([128, NPAIRS * DIM], F32R, tag="wo")
        ident = pers.tile([128, 128], F32R, tag="ident")

        nc.sync.dma_start(out=ident[:, :], in_=identd)
        for p in range(NPAIRS):
            nc.sync.dma_start(
                out=wo_sb[:, p * DIM:(p + 1) * DIM],
                in_=wo[p * 128:(p + 1) * 128, :],
            )
        # ones columns of the V|1 tiles: [:, jc*128+64 : jc*128+128] for all jc
        ones3 = onesd.rearrange("p (jc c) -> p jc c", c=64)[:, :, 0:32]
        for h in range(HPC):
            v3 = v1[h].rearrange("p (jc c) -> p jc c", c=96)
            nc.sync.dma_start(out=v3[:, :, 64:96], in_=ones3)

        # ---- phase 1: QKV projections (+ V transpose) ----
        for _rep in range(reps):
            build_phases(nc, tc, locals())


def build_phases(nc, tc, env):
    (qt, kt, v1, nt, wo_sb, ident, ppool, ypool, xT, maskrep, wq, wk, wv, wo, y,
     onesd, identd) = (
        env["qt"], env["kt"], env["v1"], env["nt"], env["wo_sb"], env["ident"],
        env["ppool"], env["ypool"], env["xT"], env["maskrep"], env["wq"],
        env["wk"], env["wv"], env["wo"], env["y"], env["onesd"], env["identd"])
    if True:
        with (
            tc.tile_pool(name="xw", bufs=1) as xw,
            tc.tile_pool(name="psproj", bufs=3, space="PSUM") as psproj,
            tc.tile_pool(name="pstp", bufs=2, space="PSUM") as pstp,
        ):
            xts = xw.tile([128, KC * N], F32R, tag="xts")
            mk = xw.tile([128, N], F32, tag="mask")
            w_sb = {}
            for name in ("q", "k", "v"):
                w_sb[name] = xw.tile([128, KC * HPC * DH], F32R,
                                     tag=f"w{name}", name=f"w{name}")
            # weights + x chunk-interleaved so the kc0 matmuls start early
            for kc in range(KC):
                for name, w in (("q", wq), ("k", wk), ("v", wv)):
                    nc.sync.dma_start(
                        out=w_sb[name][:, kc * HPC * DH:(kc + 1) * HPC * DH],
                        in_=w[kc * 128:(kc + 1) * 128, :],
                    )
                nc.sync.dma_start(
                    out=xts[:, kc * N:(kc + 1) * N],
                    in_=xT[kc * 128:(kc + 1) * 128, :],
                )
                if kc == 0:
                    nc.sync.dma_start(out=mk[:, :], in_=maskrep)

            # prefetch the exp table while the PE is projecting
            warm = ppool.tile([1, 16], F32, tag="warm", name="warm", bufs=1)
            nc.vector.memset(warm[:, :], 0.0)
            nc.scalar.activation(warm[:, :], warm[:, :], EXP)

            vt = [xw.tile([128, N], F32R, tag=f"vt{p}", name=f"vt{p}") for p in range(NPAIRS)]

            for p in range(NPAIRS):
                for name, dest in (("q", qt[p]), ("k", kt[p]), ("v", vt[p])):
                    w_t = w_sb[name]
                    for nch in range(N // 512):
                        ps = psproj.tile([128, 512], F32, tag="proj")
                        for kc in range(KC):
                            lhsT = w_t[:, kc * HPC * DH + p * 128:
                                       kc * HPC * DH + (p + 1) * 128]
                            rhs = xts[:, kc * N + nch * 512:
                                      kc * N + nch * 512 + 512]
                            nc.tensor.matmul(ps[:, :], r(lhsT), r(rhs),
                                             start=(kc == 0), stop=(kc == KC - 1))
                        dsl = dest[:, nch * 512:(nch + 1) * 512]
                        if name == "k":
                            nc.vector.tensor_mul(
                                dsl, ps[:, :], mk[:, nch * 512:(nch + 1) * 512])
                        else:
                            nc.vector.tensor_copy(dsl, ps[:, :])
                # transpose V pair into per-head V|1 tiles
                for jc in range(JC):
                    tp = pstp.tile([128, 128], F32R, tag="tp")
                    nc.tensor.transpose(
                        tp[:, :], vt[p][:, jc * 128:(jc + 1) * 128], ident[:, :])
                    nc.vector.tensor_copy(
                        v1[2 * p][:, jc * 96:jc * 96 + 64], tp[:, 0:64])
                    nc.vector.tensor_copy(
                        v1[2 * p + 1][:, jc * 96:jc * 96 + 64], tp[:, 64:128])

        # ---- phase 2: attention + output projection ----
        with (
            tc.tile_pool(name="pss", bufs=2, space="PSUM") as pss,
            tc.tile_pool(name="pso", bufs=1, space="PSUM") as pso,
            tc.tile_pool(name="psy", bufs=2, space="PSUM") as psy,
        ):
            for iq in range(IQ):
                isl = slice(iq * IQW, (iq + 1) * IQW)
                for p in range(NPAIRS):
                    oA = pso.tile([128, IQW], F32, tag="oA")
                    oB = pso.tile([128, IQW], F32, tag="oB")
                    for jc in range(JC):
                        jsl = slice(jc * 128, (jc + 1) * 128)
                        spair = pss.tile([128, 2 * IQW], F32, tag="s")
                        nc.tensor.matmul(
                            spair[:, 0:IQW],
                            r(kt[p][0:64, jsl]), r(qt[p][0:64, isl]),
                            start=True, stop=True)
                        nc.tensor.matmul(
                            spair[:, IQW:2 * IQW],
                            r(kt[p][64:128, jsl]), r(qt[p][64:128, isl]),
                            start=True, stop=True)
                        pt = ppool.tile([128, 2 * IQW], F32R, tag="pt")
                        nc.scalar.activation(pt[:, :], spair[:, :], EXP)
                        vsl = slice(jc * 96, (jc + 1) * 96)
                        nc.tensor.matmul(
                            oA[0:96, :], r(v1[2 * p][:, vsl]), r(pt[:, 0:IQW]),
                            start=(jc == 0), stop=(jc == JC - 1))
                        nc.tensor.matmul(
                            oB[0:96, :], r(v1[2 * p + 1][:, vsl]), r(pt[:, IQW:2 * IQW]),
                            start=(jc == 0), stop=(jc == JC - 1))
                    d = ppool.tile([64, IQW], F32, tag="d", name="d", bufs=2)
                    nc.vector.reciprocal(out=d[0:32, :], in_=oA[64:96, :])
                    nc.vector.reciprocal(out=d[32:64, :], in_=oB[64:96, :])
                    nc.vector.tensor_mul(
                        nt[p][0:32, isl], oA[0:32, :], d[0:32, :])
                    nc.vector.tensor_mul(
                        nt[p][32:64, isl], oA[32:64, :], d[0:32, :])
                    nc.vector.tensor_mul(
                        nt[p][64:96, isl], oB[0:32, :], d[32:64, :])
                    nc.vector.tensor_mul(
                        nt[p][96:128, isl], oB[32:64, :], d[32:64, :])
                # output projection for this query stripe
                for ib in range(IQW // 128):
                    iblk = iq * (IQW // 128) + ib
                    bsl = slice(iblk * 128, (iblk + 1) * 128)
                    ysb = ypool.tile([128, DIM], F32, tag="y")
                    for nch in range(DIM // 512):
                        yp = psy.tile([128, 512], F32, tag="yp")
                        for p in range(NPAIRS):
                            nc.tensor.matmul(
                                yp[:, :],
                                r(nt[p][:, bsl]),
                                r(wo_sb[:, p * DIM + nch * 512:
                                        p * DIM + nch * 512 + 512]),
                                start=(p == 0), stop=(p == NPAIRS - 1))
                        nc.vector.tensor_copy(
                            ysb[:, nch * 512:(nch + 1) * 512], yp[:, :])
                    nc.sync.dma_start(out=y[bsl, :], in_=ysb[:, :])


_COMPILED = None


def get_compiled(reps=1):
    global _COMPILED
    if _COMPILED is None or getattr(_COMPILED, "_reps", 1) != reps:
        nc = bacc.Bacc("TRN2", target_bir_lowering=False, debug=False,
                       num_devices=NCORES)
        with tile.TileContext(nc) as tc:
            build_kernel(nc, tc, reps=reps)
        nc.compile()
        nc._reps = reps
        _COMPILED = nc
    return _COMPILED


def make_in_maps(x, seq_mask, w_qkv, w_out):
    x = np.asarray(x, np.float32)
    seq_mask = np.asarray(seq_mask, np.float32)
    w_qkv = np.asarray(w_qkv, np.float32)
    w_out = np.asarray(w_out, np.float32)
    in_maps = []
    for c in range(NCORES):
        bc, g = divmod(c, NCORES // B)
        h0 = g * HPC * DH
        in_maps.append({
            "xT": np.ascontiguousarray(x[bc].T),
            "maskrep": np.ascontiguousarray(
                np.broadcast_to(seq_mask[bc], (128, N))),
            "wq": np.ascontiguousarray(w_qkv[:, h0:h0 + HPC * DH] * SCALE),
            "wk": np.ascontiguousarray(w_qkv[:, DIM + h0:DIM + h0 + HPC * DH]),
            "wv": np.ascontiguousarray(
                w_qkv[:, 2 * DIM + h0:2 * DIM + h0 + HPC * DH]),
            "wo": np.ascontiguousarray(w_out[h0:h0 + HPC * DH, :]),
            "identd": np.eye(128, dtype=np.float32),
            "onesd": np.ones((128, JC * 64), dtype=np.float32),
        })
    return in_maps


LAST_RESULTS = None


def kernel(x, seq_mask, w_qkv, w_out, _trace=False, **trace_kwargs):
    global LAST_RESULTS
    nc = get_compiled()
    in_maps = make_in_maps(x, seq_mask, w_qkv, w_out)
    res = bass_utils.run_bass_kernel_spmd(
        nc, in_maps, core_ids=list(range(NCORES)), trace=_trace, **trace_kwargs)
    LAST_RESULTS = res
    out = np.zeros((B, N, DIM), np.float32)
    for c in range(NCORES):
        out[c // (NCORES // B)] += res.results[c]["y"]
    return out
